# revision 28
# baseline (speedup 1.0000x reference)
"""MoE FFN (8 experts, top-2) on 8 TRN2 NeuronCores — ff-sharded + mixed fp8.

Strategy (v8):
  - Host: fp64 gate + top-2 softmax exactly like the reference. Each
    (token, slot) pair whose combine weight w < THETA is computed in fp8
    (e4m3, DoubleRow 2x matmuls); the rest in bf16. Quantization error is
    scaled by w in the combine, so small-weight pairs tolerate fp8.
  - Device (SPMD, identical program on all 8 cores): core i holds the ff
    slice [i*512:(i+1)*512) of ALL experts' W1/W2 in both precisions.
    Work list = 16 jobs (expert x precision). For every job tile it
    computes h = relu(W1s^T x + b1s) and partial y^T = W2s^T h.
    fp8 jobs: DoubleRow matmuls (2 k-tiles per instr), h quantized to
    e4m3 by the Scalar (activation) engine: h8 = relu(psum*s + SH*b1).
    Job 0 leads with a small 128-col tile so the critical first DMA is
    ~0.7MB instead of ~1.3MB; the smallest fp8 job runs last for a cheap
    writeback drain.
  - Host: y = sum of the 8 partial y's (f32), + b2, gate-weighted scatter.

Shapes (hardcoded): x [4,1024,1024] f32, Wg [1024,8], bg [8],
  W1 [8,1024,4096], b1 [8,4096], W2 [8,4096,1024], b2 [8,1024]
"""

import math

import ml_dtypes
import numpy as np

MODEL_DIM = 1024
DIM_FF = 4096
NUM_EXPERTS = 8
TOP_K = 2
N_CORES = 8
FF_SLICE = DIM_FF // N_CORES          # 512
FMS = FF_SLICE // 128                 # 4
DKS = MODEL_DIM // 128                # 8
DMS = MODEL_DIM // 128                # 8
FKS = FF_SLICE // 128                 # 4

BF16 = ml_dtypes.bfloat16
E4 = ml_dtypes.float8_e4m3

THETA = 0.425         # pairs with combine weight < THETA run in fp8
SX, SW, SH = 16.0, 256.0, 32.0
N_WARM = 14           # full-width warmups; plus narrow ones (see below)
N_WARM_NARROW = 8

_NC_CACHE: dict[tuple, object] = {}


def _plan_of(cnt: int, first_small: bool = False):
    """Tile plan for one job: (capacity c, tuple of tile widths)."""
    c = max(4, ((cnt + 3) // 4) * 4)
    plan = []
    if first_small and c > 384:
        plan.append(128)
        c0 = c - 128
    else:
        c0 = c
    n_tt = max(1, math.ceil(c0 / 512))
    ncol = ((c0 + 4 * n_tt - 1) // (4 * n_tt)) * 4
    rem = ncol * n_tt
    plan.extend([ncol] * n_tt)
    return sum(plan), tuple(plan)


def _build_v8_nc(jobs: tuple):
    """jobs: tuple of (mode, e, c, plan); mode 'b'(bf16) or 'f'(fp8)."""
    import concourse.mybir as mybir
    import concourse.tile as tile
    from concourse import bacc

    ctot = sum(j[2] for j in jobs)
    c16tot = sum(j[2] for j in jobs if j[0] == "b")
    c8tot = sum(j[2] for j in jobs if j[0] == "f")
    cmax16 = max([j[2] for j in jobs if j[0] == "b"], default=4)
    cmax8 = max([j[2] for j in jobs if j[0] == "f"], default=4)
    ymax = max(j[2] for j in jobs)
    has8 = c8tot > 0
    # per-job offsets into xs16 / xs8 / y column spaces; per-tile xoffs
    offs, o16, o8, oy = [], 0, 0, 0
    xoffs = []
    for mode, e, c, plan in jobs:
        if mode == "b":
            offs.append((o16, oy))
            o16 += c
        else:
            offs.append((o8, oy))
            o8 += c
        oy += c
        xo, acc = [], 0
        for ncol in plan:
            xo.append(acc)
            acc += ncol
        xoffs.append(tuple(xo))

    nc = bacc.Bacc("TRN2", target_bir_lowering=False)
    bias_d = nc.dram_tensor("bias", [128, 32], mybir.dt.float32, kind="ExternalInput")
    xs16_d = nc.dram_tensor("xs16", [128, 8 * max(c16tot, 4)], mybir.dt.bfloat16,
                            kind="ExternalInput")
    w1_d = nc.dram_tensor("w1", [128, 32768], mybir.dt.bfloat16, kind="ExternalInput")
    w2_d = nc.dram_tensor("w2", [128, 32768], mybir.dt.bfloat16, kind="ExternalInput")
    if has8:
        bias8_d = nc.dram_tensor("bias8", [128, 32], mybir.dt.float32,
                                 kind="ExternalInput")
        xs8_d = nc.dram_tensor("xs8", [128, 8 * c8tot], mybir.dt.float8e4,
                               kind="ExternalInput")
        w18_d = nc.dram_tensor("w18", [128, 32768], mybir.dt.float8e4,
                               kind="ExternalInput")
        w28_d = nc.dram_tensor("w28", [128, 32768], mybir.dt.float8e4,
                               kind="ExternalInput")
    # dm-pair-packed output: row block p holds dm=2p in the first C cols of
    # each job block and dm=2p+1 in the second C.
    y_d = nc.dram_tensor("y", [MODEL_DIM // 2, 2 * ctot], mybir.dt.bfloat16,
                         kind="ExternalOutput")

    with tile.TileContext(nc) as tc:
        with (
            tc.tile_pool(name="pers", bufs=1) as pers,
            tc.tile_pool(name="xsp", bufs=2) as xsp,
            tc.tile_pool(name="xs8p", bufs=2) as xs8p,
            tc.tile_pool(name="w1p", bufs=2) as w1p,
            tc.tile_pool(name="w2p", bufs=2) as w2p,
            tc.tile_pool(name="w18p", bufs=2) as w18p,
            tc.tile_pool(name="w28p", bufs=2) as w28p,
            tc.tile_pool(name="hp", bufs=3) as hp,
            tc.tile_pool(name="h8p", bufs=3) as h8p,
            tc.tile_pool(name="yp", bufs=2) as yp,
            tc.tile_pool(name="psp", bufs=8, space="PSUM") as psp,
        ):
            biast = pers.tile([128, 32], mybir.dt.float32, tag="bias", name="biast")
            if has8:
                bias8t = pers.tile([128, 32], mybir.dt.float32, tag="bias8",
                                   name="bias8t")

            # HAM warmup: dummy matmuls keep the PE busy during the initial
            # DMA wait so real matmuls start at full clock.
            warm_sb = pers.tile([128, 512], mybir.dt.bfloat16, tag="warm",
                                name="warm_sb")
            nc.vector.memset(warm_sb, 0)
            for i in range(N_WARM + N_WARM_NARROW):
                warm_ps = psp.tile([128, 512], mybir.dt.float32, tag="ps",
                                   name=f"warm_ps{i}")
                wide = 512 if i < N_WARM else 128
                nc.tensor.matmul(warm_ps[:, :wide], lhsT=warm_sb[:, :128],
                                 rhs=warm_sb[:, :wide], start=True, stop=True)

            tiles = {}

            def issue_job_dmas(ji):
                mode, e, c, plan = jobs[ji]
                oxs, _ = offs[ji]
                if mode == "b":
                    w1t = w1p.tile([128, 4096], mybir.dt.bfloat16, tag="w1",
                                   name=f"w1t_{ji}")
                    xst = xsp.tile([128, 8 * cmax16], mybir.dt.bfloat16, tag="xs",
                                   name=f"xst_{ji}")
                    w2t = w2p.tile([128, 4096], mybir.dt.bfloat16, tag="w2",
                                   name=f"w2t_{ji}")
                    xsd, w1d, w2d = xs16_d, w1_d, w2_d
                else:
                    w1t = w18p.tile([128, 4096], mybir.dt.float8e4, tag="w18",
                                    name=f"w1t_{ji}")
                    xst = xs8p.tile([128, 8 * cmax8], mybir.dt.float8e4, tag="xs8",
                                    name=f"xst_{ji}")
                    w2t = w28p.tile([128, 4096], mybir.dt.float8e4, tag="w28",
                                    name=f"w2t_{ji}")
                    xsd, w1d, w2d = xs8_d, w18_d, w28_d
                if ji == 0:
                    # Critical path, by first consumption: w1 fm0 block
                    # (128KB), first xs tile, then the rest of w1.
                    nc.sync.dma_start(w1t[:, :1024], w1d[:, e * 4096:e * 4096 + 1024])
                    nc.sync.dma_start(xst[:, :8 * plan[0]],
                                      xsd[:, 8 * oxs:8 * (oxs + plan[0])])
                    nc.sync.dma_start(w1t[:, 1024:2048],
                                      w1d[:, e * 4096 + 1024:e * 4096 + 2048])
                    nc.sync.dma_start(w1t[:, 2048:4096],
                                      w1d[:, e * 4096 + 2048:(e + 1) * 4096])
                    for tt in range(1, len(plan)):
                        lo = 8 * xoffs[ji][tt]
                        nc.sync.dma_start(
                            xst[:, lo:lo + 8 * plan[tt]],
                            xsd[:, 8 * oxs + lo:8 * oxs + lo + 8 * plan[tt]])
                    nc.sync.dma_start(w2t, w2d[:, e * 4096:(e + 1) * 4096])
                else:
                    nc.sync.dma_start(w1t, w1d[:, e * 4096:(e + 1) * 4096])
                    for tt in range(len(plan)):
                        lo = 8 * xoffs[ji][tt]
                        nc.sync.dma_start(
                            xst[:, lo:lo + 8 * plan[tt]],
                            xsd[:, 8 * oxs + lo:8 * oxs + lo + 8 * plan[tt]])
                    nc.sync.dma_start(w2t, w2d[:, e * 4096:(e + 1) * 4096])
                tiles[ji] = (w1t, xst, w2t)

            def phase_a(ji, tt):
                mode, e, c, plan = jobs[ji]
                ncol = plan[tt]
                xbase = 8 * xoffs[ji][tt]
                w1t, xst, _ = tiles[ji]
                if mode == "b":
                    hts = []
                    for fm in range(FMS):
                        ps = psp.tile([128, 512], mybir.dt.float32, tag="ps",
                                      name=f"psA_{ji}_{tt}_{fm}")
                        for dk in range(DKS):
                            nc.tensor.matmul(
                                ps[:, :ncol],
                                lhsT=w1t[:, (fm * 8 + dk) * 128:(fm * 8 + dk + 1) * 128],
                                rhs=xst[:, xbase + dk * ncol:xbase + (dk + 1) * ncol],
                                start=(dk == 0),
                                stop=(dk == DKS - 1),
                            )
                        ht = hp.tile([128, 512], mybir.dt.bfloat16, tag=f"h_{fm}",
                                     name=f"ht_{ji}_{tt}_{fm}")
                        nc.vector.tensor_scalar(
                            out=ht[:, :ncol], in0=ps[:, :ncol],
                            scalar1=biast[:, e * 4 + fm:e * 4 + fm + 1], scalar2=0.0,
                            op0=mybir.AluOpType.add, op1=mybir.AluOpType.max,
                        )
                        hts.append(ht)
                    return hts
                # fp8: DoubleRow over dk pairs; h8 packed [fk block, ncol]
                h8t = h8p.tile([128, 4 * 512], mybir.dt.float8e4, tag="h8",
                               name=f"h8t_{ji}_{tt}")
                for fm in range(FMS):
                    ps = psp.tile([128, 512], mybir.dt.float32, tag="ps",
                                  name=f"psA_{ji}_{tt}_{fm}")
                    for dkp in range(DKS // 2):
                        nc.tensor.matmul(
                            ps[:, :ncol],
                            lhsT=w1t[:, (fm * 8 + 2 * dkp) * 128:
                                     (fm * 8 + 2 * dkp + 2) * 128].rearrange(
                                         "p (k c) -> p k c", k=2),
                            rhs=xst[:, xbase + 2 * dkp * ncol:
                                    xbase + (2 * dkp + 2) * ncol].rearrange(
                                        "p (k n) -> p k n", k=2),
                            start=(dkp == 0),
                            stop=(dkp == DKS // 2 - 1),
                            perf_mode=mybir.MatmulPerfMode.DoubleRow,
                        )
                    # h8 = e4m3(relu(ps * SH/(SX*SW) + SH*b1))
                    nc.scalar.activation(
                        h8t[:, fm * 512:fm * 512 + ncol], ps[:, :ncol],
                        mybir.ActivationFunctionType.Relu,
                        bias=bias8t[:, e * 4 + fm:e * 4 + fm + 1],
                        scale=float(SH / (SX * SW)),
                    )
                return h8t

            def b_matmuls(ji, tt, hts, dm, ps):
                mode, e, c, plan = jobs[ji]
                ncol = plan[tt]
                _, _, w2t = tiles[ji]
                if mode == "b":
                    for fk in range(FKS):
                        nc.tensor.matmul(
                            ps[:, :ncol],
                            lhsT=w2t[:, (dm * 4 + fk) * 128:(dm * 4 + fk + 1) * 128],
                            rhs=hts[fk][:, :ncol],
                            start=(fk == 0),
                            stop=(fk == FKS - 1),
                        )
                else:
                    h8v = hts.rearrange("p (f n) -> p f n", f=4)
                    for fkp in range(FKS // 2):
                        nc.tensor.matmul(
                            ps[:, :ncol],
                            lhsT=w2t[:, (dm * 4 + 2 * fkp) * 128:
                                     (dm * 4 + 2 * fkp + 2) * 128].rearrange(
                                         "p (k c) -> p k c", k=2),
                            rhs=h8v[:, 2 * fkp:2 * fkp + 2, :ncol],
                            start=(fkp == 0),
                            stop=(fkp == FKS // 2 - 1),
                            perf_mode=mybir.MatmulPerfMode.DoubleRow,
                        )

            def write_yop(ji, tt, dm, ps, yops, stream):
                mode, e, c, plan = jobs[ji]
                ncol = plan[tt]
                _, oy = offs[ji]
                p, half = dm // 2, dm % 2
                lo = half * c + xoffs[ji][tt]
                merged = not isinstance(yops, list)   # last job: one 3D tile
                dst = yops[:, p, lo:lo + ncol] if merged else yops[p][:, lo:lo + ncol]
                if mode == "b":
                    nc.vector.tensor_scalar_add(dst, ps[:, :ncol], 0.0)
                elif dm % 2 == 0:
                    nc.vector.tensor_scalar_mul(dst, ps[:, :ncol],
                                                float(1.0 / (SH * SW)))
                else:
                    # split the psum drain across Vector and Scalar so the
                    # final fp8 jobs' writeback runs on two engines in parallel
                    nc.scalar.activation(
                        dst, ps[:, :ncol], mybir.ActivationFunctionType.Copy,
                        bias=0.0, scale=float(1.0 / (SH * SW)))
                if tt == len(plan) - 1:
                    if merged and dm == DMS - 1:
                        # single trigger for all 4 dm-pair blocks
                        nc.sync.dma_start(
                            y_d[:, 2 * oy:2 * oy + 2 * c].rearrange(
                                "(q p) n -> p q n", q=4),
                            yops[:, :, :2 * c])
                    elif not merged and half == 1:
                        nc.sync.dma_start(
                            y_d[p * 128:(p + 1) * 128, 2 * oy:2 * oy + 2 * c],
                            yops[p][:, :2 * c])

            def phase_b(ji, tt, hts, yops):
                for dm in range(DMS):
                    ps = psp.tile([128, 512], mybir.dt.float32, tag="ps",
                                  name=f"psB_{ji}_{tt}_{dm}")
                    b_matmuls(ji, tt, hts, dm, ps)
                    write_yop(ji, tt, dm, ps, yops, stream=False)

            # flat tile list; software pipeline A(i+1) then B(i)
            tlist = [(ji, tt) for ji in range(len(jobs))
                     for tt in range(len(jobs[ji][3]))]
            issue_job_dmas(0)
            nc.sync.dma_start(biast, bias_d[:, :])
            if has8:
                nc.sync.dma_start(bias8t, bias8_d[:, :])
            if len(jobs) > 1:
                issue_job_dmas(1)

            yops_of = {}
            hts_of = {}
            for idx, (ji, tt) in enumerate(tlist):
                if tt == 0:
                    if ji + 2 < len(jobs):
                        issue_job_dmas(ji + 2)
                    if ji == len(jobs) - 1:
                        c_last = jobs[ji][2]
                        yops_of[ji] = yp.tile(
                            [128, 4, 2 * c_last], mybir.dt.bfloat16,
                            tag="yopL", name=f"yopL_{ji}")
                    else:
                        yops_of[ji] = [
                            yp.tile([128, 2 * ymax], mybir.dt.bfloat16,
                                    tag=f"yop_{p}", name=f"yop_{ji}_{p}")
                            for p in range(DMS // 2)]
                hts_of[(ji, tt)] = phase_a(ji, tt)
                if idx > 0:
                    pji, ptt = tlist[idx - 1]
                    phase_b(pji, ptt, hts_of.pop((pji, ptt)), yops_of[pji])
                    if ptt == len(jobs[pji][3]) - 1:
                        del tiles[pji]
            lji, ltt = tlist[-1]
            phase_b(lji, ltt, hts_of.pop((lji, ltt)), yops_of[lji])

    nc.compile()
    return nc


def _route_host(x, Wg, bg):
    """Reference-exact gate + fp8/bf16 assignment.

    Returns per expert: (tok16, w16, tok8, w8)."""
    T = x.shape[0]
    logits = x.astype(np.float64) @ Wg.astype(np.float64) + bg.astype(np.float64)
    order = np.argsort(-logits, axis=1, kind="stable")[:, :TOP_K]  # [T, 2]
    vals = np.take_along_axis(logits, order, axis=1)
    vmax = vals.max(axis=1, keepdims=True)
    ev = np.exp(vals - vmax)
    w = (ev / ev.sum(axis=1, keepdims=True)).astype(np.float32)  # [T, 2]
    tok = np.repeat(np.arange(T), TOP_K)
    exp = order.ravel()
    wgt = w.ravel()
    is8 = wgt < THETA
    out = []
    for e in range(NUM_EXPERTS):
        m16 = (exp == e) & ~is8
        m8 = (exp == e) & is8
        out.append((tok[m16], wgt[m16], tok[m8], wgt[m8]))
    return out


def _jobs_of(route):
    jobs = []
    members = []
    for e in range(NUM_EXPERTS):
        tok16, w16, tok8, w8 = route[e]
        if len(tok16) or not len(tok8):
            jobs.append(["b", e, len(tok16)])
            members.append((tok16, w16))
        if len(tok8):
            jobs.append(["f", e, len(tok8)])
            members.append((tok8, w8))
    # End with the three smallest fp8 jobs (smallest very last): the final
    # writeback drain is tiny and the preceding big bf16 job's y DMA hides
    # behind ~9us of fp8 compute.
    fidx = sorted((i for i, j in enumerate(jobs) if j[0] == "f"),
                  key=lambda i: jobs[i][2], reverse=True)
    tail_ids = fidx[-3:] if len(fidx) >= 3 else fidx
    if tail_ids:
        order = [i for i in range(len(jobs)) if i not in tail_ids] + tail_ids
        jobs = [jobs[i] for i in order]
        members = [members[i] for i in order]
    out = []
    for mode, e, cnt in jobs:
        c, plan = _plan_of(cnt)
        out.append((mode, e, c, plan))
    return tuple(out), members


def _pack_xs(xt, jobs, members):
    """xs16 [128, 8*c16tot] bf16 and xs8 [128, 8*c8tot] e4m3 (x*SX)."""
    c16tot = sum(j[2] for j in jobs if j[0] == "b")
    c8tot = sum(j[2] for j in jobs if j[0] == "f")
    xs16 = np.zeros((128, 8 * max(c16tot, 4)), dtype=BF16)
    xs8 = np.zeros((128, 8 * max(c8tot, 4)), dtype=E4)
    o16 = o8 = 0
    for (mode, e, c, plan), (tokens, _) in zip(jobs, members):
        cnt = len(tokens)
        if mode == "b":
            xT = np.zeros((MODEL_DIM, c), dtype=BF16)
            if cnt:
                xT[:, :cnt] = xt[tokens].astype(BF16).T
            dst, off = xs16, o16
            o16 += 8 * c
        else:
            xT = np.zeros((MODEL_DIM, c), dtype=E4)
            if cnt:
                xT[:, :cnt] = (xt[tokens].astype(np.float32) * SX).astype(E4).T
            dst, off = xs8, o8
            o8 += 8 * c
        xo = 0
        for ncol in plan:
            for dk in range(8):
                dst[:, off + 8 * xo + dk * ncol: off + 8 * xo + (dk + 1) * ncol] = \
                    xT[dk * 128:(dk + 1) * 128, xo:xo + ncol]
            xo += ncol
    return np.ascontiguousarray(xs16), np.ascontiguousarray(xs8)


def _pack_weights(W1, W2, b1):
    """Per-core packed ff-slices in bf16 and e4m3.

    w1_packs[i] [128, 32768]: block (e, fm, dk) at ((e*4+fm)*8+dk)*128 =
      W1[e][dk*128:(dk+1)*128, i*512+fm*128 : i*512+(fm+1)*128]
    w2_packs[i] [128, 32768]: block (e, dm, fk) at ((e*8+dm)*4+fk)*128 =
      W2[e][i*512+fk*128 : i*512+(fk+1)*128, dm*128:(dm+1)*128]
    bias_packs[i] [128, 32] f32: col e*4+fm = b1[e][i*512+fm*128 : +128]
    """
    W1b = W1.reshape(8, 8, 128, 8, 4, 128)                    # e,dk,r,i,fm,c
    W1t = W1b.transpose(3, 2, 0, 4, 1, 5).reshape(8, 128, 32768)
    w1_packs = np.ascontiguousarray(W1t.astype(BF16))
    w18_packs = np.ascontiguousarray((W1t * SW).astype(E4))
    W2b = W2.reshape(8, 8, 4, 128, 8, 128)                    # e,i,fk,r,dm,c
    W2t = W2b.transpose(1, 3, 0, 4, 2, 5).reshape(8, 128, 32768)
    w2_packs = np.ascontiguousarray(W2t.astype(BF16))
    w28_packs = np.ascontiguousarray((W2t * SW).astype(E4))
    b1b = b1.astype(np.float32).reshape(8, 8, 4, 128)         # e,i,fm,r
    bias_packs = np.ascontiguousarray(b1b.transpose(1, 3, 0, 2).reshape(8, 128, 32))
    bias8_packs = np.ascontiguousarray(bias_packs * SH)
    return w1_packs, w2_packs, w18_packs, w28_packs, bias_packs, bias8_packs


def _prepare(x, Wg, bg, W1, b1, W2, b2):
    B, S, d = x.shape
    T = B * S
    xt = x.reshape(T, d)

    route = _route_host(xt, Wg, bg)
    jobs, members = _jobs_of(route)

    if jobs not in _NC_CACHE:
        _NC_CACHE[jobs] = _build_v8_nc(jobs)
    nc = _NC_CACHE[jobs]

    xs16, xs8 = _pack_xs(xt, jobs, members)
    w1p, w2p, w18p, w28p, bp, b8p = _pack_weights(W1, W2, b1)
    has8 = any(j[0] == "f" for j in jobs)
    in_maps = []
    for i in range(N_CORES):
        m = {"bias": bp[i], "xs16": xs16, "w1": w1p[i], "w2": w2p[i]}
        if has8:
            m.update({"bias8": b8p[i], "xs8": xs8, "w18": w18p[i],
                      "w28": w28p[i]})
        in_maps.append(m)
    return nc, in_maps, jobs, members


def kernel(x, Wg, bg, W1, b1, W2, b2):
    from concourse.bass_utils import run_bass_kernel_spmd

    B, S, d = x.shape
    T = B * S
    nc, in_maps, jobs, members = _prepare(x, Wg, bg, W1, b1, W2, b2)
    res = run_bass_kernel_spmd(nc, in_maps, core_ids=list(range(N_CORES)))

    ctot = sum(j[2] for j in jobs)
    ypacked = res.results[0]["y"].astype(np.float32)  # [512, 2*ctot]
    for i in range(1, N_CORES):
        ypacked += res.results[i]["y"].astype(np.float32)
    # unpack dm-pair layout -> ysum [1024, ctot]
    ysum = np.empty((MODEL_DIM, ctot), dtype=np.float32)
    off = 0
    for mode, e, c, plan in jobs:
        blk = ypacked[:, 2 * off:2 * off + 2 * c]
        for p in range(4):
            ysum[2 * p * 128:(2 * p + 1) * 128, off:off + c] = \
                blk[p * 128:(p + 1) * 128, :c]
            ysum[(2 * p + 1) * 128:(2 * p + 2) * 128, off:off + c] = \
                blk[p * 128:(p + 1) * 128, c:]
        off += c

    out = np.zeros((T, d), dtype=np.float32)
    off = 0
    for (mode, e, c, plan), (tokens, wgt) in zip(jobs, members):
        cnt = len(tokens)
        if cnt:
            ye = ysum[:, off:off + cnt].T + b2[e].astype(np.float32)
            out[tokens] += ye * wgt[:, None]
        off += c
    return out.reshape(B, S, d)
